# revision 24
# baseline (speedup 1.0000x reference)
"""Trainium2 Bass kernel: 3x3 conv (stride 1, pad 1) via shifted-matmul, bf16.

Full problem: x (32, 18, 256, 256) f32, weight (64, 18, 3, 3), bias (64,)
-> out (32, 64, 256, 256).  Data-parallel over batch: 8 cores x 4 images.

Per-core algorithm (v2, bf16 + 4-way TensorE tile packing):
  - All matmul data is bf16 (tolerance is 2e-2; bf16 lands ~3e-3), which
    halves HBM traffic vs f32 and unlocks 64x64 array tiling (fp32r
    requires PSUM partition 0, pinning everything to one array quadrant).
  - Two "lanes": SBUF partitions 0..53 (lane 0, images 0..1) and 64..117
    (lane 1, images 2..3), each holding a G strip buffer [54, R, 258]
    with partition p = 3c + g (kh-group g of channel c) and a copy of
    the weights.  K=54 <= 64 and M=64 <= 64, so each matmul occupies one
    64x64 quadrant of the PE array; the four (SBUF half x PSUM half)
    quadrants T0/T2/T8/T10 run concurrently when consecutive matmuls
    rotate across them.
  - Per strip, 2 rounds of 8 PSUM banks [128, 512]: bank = 2 output row
    pairs (partitions 0:64 <- array cols 0:64, rows 8q+2b; partitions
    64:128 <- cols 64:128, rows 16+8q+2b).  3 accumulating matmuls per
    row pair (kw taps as rhs column offsets).
  - Drain: one ACT-or-DVE op per bank [128, 512] f32 -> bf16 + bias.
  - Stores: per lane-strip one [128, 8, 512] bf16 staging tile -> 1 MB
    DMA with 8 KB/partition contiguous runs (scalar HWDGE ring); strip
    loads (0.9 MB, 16.5 KB/partition runs) ride the sync HWDGE ring.
"""

import re
import numpy as np

import bass_rust
import concourse.bass as bass
import concourse.mybir as mybir
from concourse.tile import TileContext


# ---------------------------------------------------------------------------
# TileContext drain patch: this walrus build rejects an InstDrain carrying
# more than ~2 sync waits ("Too many sync wait commands").  Re-emit the
# end-of-kernel global-clock waits as one nop per semaphore, then drain.
# ---------------------------------------------------------------------------
def _patched_drain_and_barrier(self, tick_clock, wait_clock):
    gc = tick_clock.global_clock
    vals = [int(s) for s in re.findall(r"\d+", repr(gc))]
    for i, v in enumerate(vals):
        if v > 0:
            c = bass_rust.VectorClock()
            c.require_at_least(i, v)
            nop = self.nc.sync.nop(nofuse=True, hint=f"drain_wait_{i}")
            wait_clock.add_sem_waits(nop.ins, bass_rust.ScopedClock({None: c}))
    self.nc.sync.drain()

    self.nc.all_engine_barrier()
    assert self.sems is not None
    popped = self.nc._tile_sem_poison_stack.pop()
    assert popped is self._sem_poison
    self.nc.clear_and_free_semaphores(list(self.sems.allocated().values()))
    self.nc.all_engine_barrier()


TileContext._drain_and_barrier = _patched_drain_and_barrier


def _patch_ldw_opt(enable=True):
    """Toggle walrus's load-weights elision (the pipeline passes
    --enable-ldw-opt=false).  NOTE: ldw-opt rejects LDWEIGHTS with a
    nonzero column tile position, so it must stay off for kernels using
    PSUM-half column packing (tile_position[1] == 64)."""
    import concourse.bass_utils as _bu

    if getattr(_bu, "_ldw_opt_patched", None) == enable:
        return
    _orig = getattr(_bu, "_ldw_orig_run_command", _bu.run_command)
    _bu._ldw_orig_run_command = _orig

    def _patched(cmd, *a, **kw):
        if enable:
            cmd = [
                "--enable-ldw-opt=true" if c == "--enable-ldw-opt=false" else c
                for c in cmd
            ]
        return _orig(cmd, *a, **kw)

    _bu.run_command = _patched
    _bu._ldw_opt_patched = enable


def _split_excess_waits(nc, max_waits=1):
    """This walrus build allows very few sync waits per instruction.
    Hoist excess waits onto same-engine nops placed just before."""
    for f in nc.m.functions:
        for bb in f.blocks:
            out = []
            changed = False
            for inst in bb.instructions:
                si = inst.sync_info
                waits = list(si.on_wait) if si and si.on_wait else []
                if len(waits) > max_waits:
                    changed = True
                    extras, keep = waits[:-max_waits], waits[-max_waits:]
                    for j, w in enumerate(extras):
                        nop = mybir.InstNoOp(
                            name=f"{inst.name}_xw{j}", ins=[], outs=[]
                        )
                        nop.engine = inst.engine
                        nop.sync_info = mybir.SyncInfo(on_wait=[w], on_update=[])
                        out.append(nop)
                    inst.sync_info = mybir.SyncInfo(
                        on_wait=keep,
                        on_update=list(si.on_update) if si.on_update else [],
                    )
                out.append(inst)
            if changed:
                bb.instructions = out


# ---------------------------------------------------------------------------
# Kernel builder
# ---------------------------------------------------------------------------
F32 = mybir.dt.float32
BF16 = mybir.dt.bfloat16


def build_conv_nc(
    n_img=4,
    H=256,
    W=256,
    R=64,
    C_IN=18,
    C_OUT=64,
    act_frac=4,  # of 8 drains per round, how many go to ACT (rest DVE)
    mm_order="rot4",  # rot4 | rot2 | block
    store_rings=("scalar", "gpsimd"),  # per-lane store ring
):
    """Build the per-core Bass program. Returns nc."""
    assert H % R == 0 and R % 16 == 0 and n_img % 2 == 0
    Wp = W + 2
    Hp = H + 2
    GP = 3 * C_IN  # 54 partitions per lane
    LH = 64  # lane-1 base partition

    nc = bass.Bass()
    # x is host-pre-padded AND pre-replicated into the G layout: row
    # 3c+g of x3 = padded channel c shifted up by g rows.  Strip loads
    # become a single affine partition dim of 54, which the DMA splitter
    # can spread over all 16 SDMA engines (an 18-channel outer dim only
    # reaches engine slots 0-8).
    x = nc.dram_tensor("x", [n_img, GP, Hp, Wp], BF16, kind="ExternalInput")
    wT = nc.dram_tensor("wT", [LH + GP, 3, C_OUT], BF16, kind="ExternalInput")
    bias2 = nc.dram_tensor("bias2", [2 * C_OUT, 1], F32, kind="ExternalInput")
    y = nc.dram_tensor("y", [n_img, C_OUT, H, W], BF16, kind="ExternalOutput")

    # Strip schedule: small strips at the kernel's start and end shorten
    # the pipeline fill (first matmul waits on one strip load) and drain
    # (last store waits on the final strip), 64-row strips in steady
    # state for big DMA descriptor runs.
    if R == 64 and H == 256:
        sched = (16, 16, 64, 64, 64, 16, 16)
    else:
        sched = (R,) * (H // R)
    assert sum(sched) == H
    n_strips = len(sched)
    half = n_img // 2
    x_ap = x[:]
    y_ap = y[:]

    ring = {
        "scalar": nc.scalar,
        "sync": nc.sync,
        "gpsimd": nc.gpsimd,
    }

    with TileContext(nc) as tc:
        with (
            tc.tile_pool(name="wpool", bufs=1) as wpool,
            tc.tile_pool(name="gpool", bufs=4) as gpool,
            tc.tile_pool(name="opool", bufs=4) as opool,
            tc.tile_pool(name="psum", bufs=8, space="PSUM") as pspool,
        ):
            wsb = wpool.tile([LH + GP, 3, C_OUT], BF16, tag="wsb")
            bsb = wpool.tile([2 * C_OUT, 1], F32, tag="bsb")
            nc.sync.dma_start(out=wsb[:, :, :], in_=wT[:])
            nc.sync.dma_start(out=bsb[:], in_=bias2[:])

            tile_idx = 0
            for n2 in range(half):
                imgs = (n2, half + n2)
                h0 = 0
                for s in range(n_strips):
                    Rs = sched[s]
                    if s > 0:
                        h0 += sched[s - 1]
                    rounds = Rs // 16
                    final_strip = n2 == half - 1 and s == n_strips - 1
                    G_t = gpool.tile([LH + GP, Rs, Wp], BF16, tag="G", name="G")
                    # Two DMAs per lane (32+22 partitions, ranges
                    # alternating per strip): outer dims 32/22 chunk to
                    # 16/11 engine slots, balancing load bytes across
                    # all 16 SDMA engines over consecutive strips.
                    if (n2 * n_strips + s) % 2 == 0:
                        splits = ((0, 32), (32, GP))
                    else:
                        splits = ((0, 22), (22, GP))
                    for li, n in enumerate(imgs):
                        gb = li * LH
                        for p0, p1 in splits:
                            src = bass.AP(
                                tensor=x_ap.tensor,
                                offset=(n * GP + p0) * Hp * Wp + h0 * Wp,
                                ap=[[Hp * Wp, p1 - p0], [1, Rs * Wp]],
                            )
                            nc.sync.dma_start(
                                out=G_t[gb + p0 : gb + p1], in_=src
                            )
                    OBs = [
                        opool.tile(
                            [2 * C_OUT, Rs // 4, 512], BF16, tag="OB", name="OB"
                        )
                        for _ in range(2)
                    ]
                    for q in range(rounds):
                        PTs = [
                            [
                                pspool.tile(
                                    [2 * C_OUT, 512], F32, tag="PT", name="PT"
                                )
                                for _ in range(4)
                            ]
                            for _ in range(2)
                        ]
                        # Matmul emission order: consecutive MMs should
                        # land on different array quadrants so they run
                        # concurrently (tile packing).
                        if mm_order == "rot4":
                            quads = [
                                (li, ch, b)
                                for b in range(4)
                                for ch in (0, 1)
                                for li in (0, 1)
                            ]
                        elif mm_order == "rot2":
                            quads = [
                                (li, 0, b) for b in range(4) for li in (0, 1)
                            ] + [
                                (li, 1, b) for b in range(4) for li in (0, 1)
                            ]
                        else:  # block: runs of 4 sharing one stationary
                            quads = [
                                (li, ch, b)
                                for li in (0, 1)
                                for ch in (0, 1)
                                for b in range(4)
                            ]
                        for kw in range(3):
                            for li, ch, b in quads:
                                gb = li * LH
                                l = 8 * q + 2 * b + (Rs // 2) * ch
                                nc.tensor.matmul(
                                    PTs[li][b][64 * ch : 64 * ch + 64],
                                    wsb[gb : gb + GP, kw, :],
                                    G_t[gb : gb + GP, l : l + 2, kw : kw + W],
                                    start=(kw == 0),
                                    stop=(kw == 2),
                                    skip_group_check=True,
                                )
                        # Drain: one op per bank [128, 512] (f32 PSUM ->
                        # bf16 staging + bias), split across ACT and DVE.
                        for li in range(2):
                            for b in range(4):
                                PT = PTs[li][b]
                                dst = OBs[li][:, 4 * q + b, :]
                                if tile_idx % 8 < act_frac:
                                    nc.scalar.activation(
                                        dst,
                                        PT[:],
                                        mybir.ActivationFunctionType.Identity,
                                        bias=bsb[0 : 2 * C_OUT],
                                    )
                                else:
                                    nc.vector.tensor_scalar_add(
                                        dst, PT[:], bsb[0 : 2 * C_OUT]
                                    )
                                tile_idx += 1
                    # Store: per lane, 2 DMAs (one per partition half g,
                    # outer dim = 64 channels so the DMA splitter engages
                    # all 16 SDMA engines).  Partition (g, c) holds rows
                    # [h0 + (R/2)g, h0 + (R/2)(g+1)) of channel c as one
                    # (R/2)*W contiguous run.
                    for li, n in enumerate(imgs):
                        for g in range(2):
                            if final_strip:
                                # Spread the kernel's last stores over
                                # all three rings (loads are done; the
                                # sync ring is free).
                                eng = ring[
                                    ("scalar", "sync", "gpsimd", "scalar")[
                                        2 * li + g
                                    ]
                                ]
                            else:
                                eng = ring[store_rings[li]]
                            dst = bass.AP(
                                tensor=y_ap.tensor,
                                offset=n * C_OUT * H * W
                                + (h0 + (Rs // 2) * g) * W,
                                ap=[[H * W, C_OUT], [1, (Rs // 2) * W]],
                            )
                            eng.dma_start(
                                out=dst, in_=OBs[li][64 * g : 64 * g + 64]
                            )
    return nc


# ---------------------------------------------------------------------------
# Host-side entry point
# ---------------------------------------------------------------------------
N_CORES = 8


def prep_inputs(x_shard, weight, bias):
    import ml_dtypes

    bf16 = ml_dtypes.bfloat16
    # lhsT row 3c+g = weight[:, c, g, kw]; lhsT col = oc.  Duplicated at
    # partitions 0..53 (lane 0) and 64..117 (lane 1).
    w54 = np.ascontiguousarray(
        np.transpose(weight, (1, 2, 3, 0)).reshape(54, 3, 64)
    ).astype(bf16)
    wT = np.zeros((118, 3, 64), bf16)
    wT[0:54] = w54
    wT[64:118] = w54
    bias2 = np.concatenate([bias, bias]).reshape(128, 1).astype(np.float32)
    n, c, H, W = x_shard.shape
    Hp, Wp = H + 2, W + 2
    x_pad = np.zeros((n, c, Hp, Wp), bf16)
    x_pad[:, :, 1 : H + 1, 1 : W + 1] = x_shard.astype(bf16)
    # Pre-replicate into the G layout: x3[n, 3c+g, r, :] = x_pad[n, c,
    # r+g, :] (kh-group g baked in as a row shift; tail rows unused).
    x3 = np.zeros((n, 3 * c, Hp, Wp), bf16)
    for g in range(3):
        x3[:, g::3, : Hp - g, :] = x_pad[:, :, g:, :]
    return {"x": x3, "wT": wT, "bias2": bias2}


def run(x, weight, bias, trace=False, ldw_opt=False, **build_kwargs):
    from concourse.bass_utils import run_bass_kernel_spmd

    x = np.asarray(x, dtype=np.float32)
    weight = np.asarray(weight, dtype=np.float32)
    bias = np.asarray(bias, dtype=np.float32)

    B = x.shape[0]
    per = B // N_CORES
    nc = build_conv_nc(n_img=per, **build_kwargs)
    _split_excess_waits(nc)
    _patch_ldw_opt(ldw_opt)
    in_maps = [
        prep_inputs(x[i * per : (i + 1) * per], weight, bias)
        for i in range(N_CORES)
    ]
    res = run_bass_kernel_spmd(nc, in_maps, list(range(N_CORES)), trace=trace)
    y = np.concatenate(
        [np.asarray(res.results[i]["y"], dtype=np.float32) for i in range(N_CORES)],
        axis=0,
    )
    return y, res


def kernel(x, weight, bias):
    return run(x, weight, bias)[0]
